# revision 13
# baseline (speedup 1.0000x reference)
"""Trainium2 Bass kernel: 4096x4096 valid cross-correlation with an 11x11
filter + scalar bias, sharded row-wise across 8 NeuronCores.

Strategy
--------
Host-side sharding (halo = overlapping row slices, no collectives): core m
gets input rows [512m, 512m + 522) (core 7 shifted up to stay in bounds)
and produces output rows [512m, 512m + 512).

Per-core compute: conv expressed as banded matmuls on the TensorEngine.
For each kernel column dj, a banded stationary matrix
    B_dj[k, m] = w[k - m, dj]   (0 <= k - m < 11)
contracts over 128 image rows, while column-shifted slices of the image
slab stream as the moving operand:
    out[m, n] += sum_k B_dj[k, m] * x[r0 + k, n0 + n + dj]
Accumulating the 11 dj-shifted matmuls in one PSUM bank yields the full
11x11 correlation for a [118, 512] output tile. float32r runs the PE at
1 cycle/row (vs 4 for plain fp32) with fp32 operands.
"""

import os
import sys

import numpy as np

for _p in ("/opt/trn_rl_repo", "/root/.axon_site/_ro/trn_rl_repo"):
    if os.path.isdir(_p) and _p not in sys.path:
        sys.path.insert(0, _p)

# The device run goes through jax's axon PJRT backend; make sure it is
# visible if jax has not been initialized yet.
_jp = os.environ.get("JAX_PLATFORMS", "")
if "axon" not in _jp.split(","):
    os.environ["JAX_PLATFORMS"] = ("axon," + _jp).strip(",")

import concourse.bacc as bacc
import concourse.bass as bass
import concourse.mybir as mybir
import concourse.tile as tile
from concourse.bass_utils import run_bass_kernel_spmd

H = W = 4096
KH = KW = 11
OH = OW = H - KH + 1  # 4086
NCORES = 8
ROWS_OUT = 512            # output rows per core
ROWS_IN = ROWS_OUT + KH - 1  # 522
M_FULL = 118              # output rows per full slab (contraction K = 128)
# (x row offset, out row offset, M out rows, band column offset) per slab.
# All slabs contract over K=128 input rows: the 40-row tail reads the last
# 128 slab rows (394..521) and picks the shifted band columns 78..117, so
# the PE always runs at full contraction duty (HAM stays warm).
SLABS = [(0, 0, 118, 0), (118, 118, 118, 0), (236, 236, 118, 0),
         (354, 354, 118, 0), (394, 472, 40, 78)]
BANK_N = [512] * 7 + [OW - 7 * 512]  # 7x512 + 502 = 4086

_cache: dict = {}
LAST_RESULT = None  # BassKernelResults of the most recent device run


def _build():
    f32 = mybir.dt.float32
    f32r = mybir.dt.float32r
    nc = bacc.Bacc("TRN2", target_bir_lowering=False, debug=False,
                   num_devices=NCORES)
    xs_d = nc.dram_tensor("xs", [ROWS_IN, W], f32r, kind="ExternalInput")
    bd_d = nc.dram_tensor("bands", [128, KW * M_FULL], f32r,
                          kind="ExternalInput")
    bias_d = nc.dram_tensor("biasv", [1, 1], f32, kind="ExternalInput")
    out_d = nc.dram_tensor("out", [ROWS_OUT, OW], f32, kind="ExternalOutput")

    with tile.TileContext(nc) as tc:
        with (
            tc.tile_pool(name="bp", bufs=1) as bp,
            tc.tile_pool(name="xp", bufs=1) as xp,
            tc.tile_pool(name="op", bufs=3) as op,
            tc.tile_pool(name="pp", bufs=6, space=bass.MemorySpace.PSUM) as pp,
            tc.tile_pool(name="pw", bufs=1, space=bass.MemorySpace.PSUM) as pw,
        ):
            def dma_rows(eng, dst, src, rows, nsplit):
                # split a [rows, ...] transfer into row chunks so the HWDGE
                # fans it across more DMA engines (one 16KB packet per row;
                # a single dma_start only engages ~2 engines)
                step = (rows + nsplit - 1) // nsplit
                for c0 in range(0, rows, step):
                    c1 = min(c0 + step, rows)
                    eng.dma_start(dst[c0:c1], src[c0:c1])

            # bands first on the sync queue (gates the first matmul)
            bt = bp.tile([128, KW * M_FULL], f32r, name="bt")
            dma_rows(nc.sync, bt, bd_d.ap()[:, :], 128, 4)

            # bias: one-packet DMA, then broadcast across partitions with a
            # K=1 matmul against a ones row (a 128-packet broadcast DMA is
            # ~13us; this chain is ~2us)
            bias_sb = bp.tile([1, 1], f32, name="bias_sb")
            nc.sync.dma_start(bias_sb[:], bias_d.ap()[:, :])
            ones_t = bp.tile([1, 128], f32, name="ones_t")
            nc.gpsimd.memset(ones_t[:], 1.0)
            bias_ps = pw.tile([128, 1], f32, name="bias_ps")
            nc.tensor.matmul(bias_ps[:], ones_t[:], bias_sb[:],
                             start=True, stop=True)
            bias_bc = bp.tile([128, 1], f32, name="bias_bc")
            nc.scalar.copy(bias_bc[:], bias_ps[:])

            # all slab loads issued upfront: every xt is SBUF-resident (5 x
            # 2MB), so no load trigger ever queues behind a store's wait.
            # Triggers alternate between the Sync queue and GpSimd (SWDGE)
            # so trigger issue itself is parallel; slab 0 goes first.
            xts = {}
            for si, (r0, _, _, _) in enumerate(SLABS):
                xt = xp.tile([128, W], f32r, tag=f"xt{si}", name=f"xt{si}")
                step = 16
                for ci, c0 in enumerate(range(0, 128, step)):
                    eng = nc.sync if ci % 2 == 0 else nc.gpsimd
                    eng.dma_start(xt[c0:c0 + step],
                                  xs_d.ap()[r0 + c0:r0 + c0 + step, :])
                xts[si] = xt

            # warm the PE's HAM clock gate while slab 0's DMA is in flight
            warm = pw.tile([118, 512], f32, name="warm")
            for i in range(14):
                nc.tensor.matmul(warm[:, :], bt[:, 0:118], bt[:, 0:512],
                                 start=(i == 0), stop=(i == 13))

            for si, (r0, o0, M, boff) in enumerate(SLABS):
                xt = xts[si]
                ot = op.tile([M, OW], f32, tag="ot", name=f"ot{si}")
                for b in range(8):
                    n0 = b * 512
                    N = BANK_N[b]
                    pt = pp.tile([M, 512], f32, tag="ps", name=f"ps{si}_{b}")
                    for dj in range(KW):
                        nc.tensor.matmul(
                            pt[:, :N],
                            bt[:, dj * M_FULL + boff: dj * M_FULL + boff + M],
                            xt[:, n0 + dj: n0 + dj + N],
                            start=(dj == 0),
                            stop=(dj == KW - 1),
                        )
                    nc.scalar.activation(
                        ot[:, n0:n0 + N], pt[:, :N],
                        mybir.ActivationFunctionType.Identity,
                        bias=bias_bc[0:M, :],
                    )
                # stores on sync: all load triggers were already issued, so
                # a store trigger waiting on this slab's ACTs blocks nothing
                dma_rows(nc.sync, out_d.ap()[o0:o0 + M, :], ot[:], M, 2)
    nc.compile()
    return nc


def _bands_from_weight(weight: np.ndarray) -> np.ndarray:
    b = np.zeros((128, KW * M_FULL), np.float32)
    for dj in range(KW):
        col = weight[:, dj].astype(np.float32)
        for m in range(M_FULL):
            b[m:m + KH, dj * M_FULL + m] = col
    return b


def kernel(x: np.ndarray, weight: np.ndarray, bias: np.ndarray,
           _trace: bool = False, **_trace_kwargs) -> np.ndarray:
    global LAST_RESULT
    x = np.asarray(x, dtype=np.float32)
    weight = np.asarray(weight, dtype=np.float32)
    bias_v = np.asarray(bias, dtype=np.float32).reshape(1, 1)

    if "nc" not in _cache:
        _cache["nc"] = _build()
    nc = _cache["nc"]

    bands = _bands_from_weight(weight)
    starts = [min(m * ROWS_OUT, H - ROWS_IN) for m in range(NCORES)]
    in_maps = [
        {"xs": np.ascontiguousarray(x[s:s + ROWS_IN]),
         "bands": bands,
         "biasv": bias_v}
        for s in starts
    ]
    res = run_bass_kernel_spmd(nc, in_maps, core_ids=list(range(NCORES)),
                               trace=_trace, **_trace_kwargs)
    LAST_RESULT = res

    out = np.empty((OH, OW), dtype=np.float32)
    for m, s in enumerate(starts):
        r = res.results[m]["out"]
        g0 = m * ROWS_OUT           # first global output row wanted from core m
        keep0 = g0 - s              # 0 for cores 0-6, 10 for core 7
        take = min(ROWS_OUT - keep0, OH - g0)
        out[g0:g0 + take] = r[keep0:keep0 + take]
    return out


# revision 15
# speedup vs baseline: 2.6807x; 2.6807x over previous
"""Trainium2 Bass kernel: 4096x4096 valid cross-correlation with an 11x11
filter + scalar bias, sharded row-wise across 8 NeuronCores.

Strategy
--------
Host-side sharding (halo = overlapping row slices, no collectives): core m
gets input rows [512m, 512m + 522) (core 7 shifted up to stay in bounds)
and produces output rows [512m, 512m + 512).

Per-core compute: conv expressed as banded matmuls on the TensorEngine.
For each kernel column dj, a banded stationary matrix
    B_dj[k, m] = w[k - m, dj]   (0 <= k - m < 11)
contracts over 128 image rows, while column-shifted slices of the image
slab stream as the moving operand:
    out[m, n] += sum_k B_dj[k, m] * x[r0 + k, n0 + n + dj]
Accumulating the 11 dj-shifted matmuls in one PSUM bank yields the full
11x11 correlation for a [118, 512] output tile. float32r runs the PE at
1 cycle/row (vs 4 for plain fp32) with fp32 operands.
"""

import os
import sys

import numpy as np

for _p in ("/opt/trn_rl_repo", "/root/.axon_site/_ro/trn_rl_repo"):
    if os.path.isdir(_p) and _p not in sys.path:
        sys.path.insert(0, _p)

# The device run goes through jax's axon PJRT backend; make sure it is
# visible if jax has not been initialized yet.
_jp = os.environ.get("JAX_PLATFORMS", "")
if "axon" not in _jp.split(","):
    os.environ["JAX_PLATFORMS"] = ("axon," + _jp).strip(",")

import concourse.bacc as bacc
import concourse.bass as bass
import concourse.mybir as mybir
import concourse.tile as tile
from concourse.bass_utils import run_bass_kernel_spmd

H = W = 4096
KH = KW = 11
OH = OW = H - KH + 1  # 4086
NCORES = 8
ROWS_OUT = 512            # output rows per core
ROWS_IN = ROWS_OUT + KH - 1  # 522
M_FULL = 118              # output rows per full slab (contraction K = 128)
# (x row offset, out row offset, M out rows, band column offset) per slab.
# All slabs contract over K=128 input rows: the 40-row tail reads the last
# 128 slab rows (394..521) and picks the shifted band columns 78..117, so
# the PE always runs at full contraction duty (HAM stays warm).
SLABS = [(0, 0, 118, 0), (118, 118, 118, 0), (236, 236, 118, 0),
         (354, 354, 118, 0), (394, 472, 40, 78)]
BANK_N = [512] * 7 + [OW - 7 * 512]  # 7x512 + 502 = 4086

_cache: dict = {}
LAST_RESULT = None  # BassKernelResults of the most recent device run


def _build():
    f32 = mybir.dt.float32
    f32r = mybir.dt.float32r
    nc = bacc.Bacc("TRN2", target_bir_lowering=False, debug=False,
                   num_devices=NCORES)
    xs_d = nc.dram_tensor("xs", [ROWS_IN, W], f32r, kind="ExternalInput")
    bd_d = nc.dram_tensor("bands", [128, KW * M_FULL], f32r,
                          kind="ExternalInput")
    bias_d = nc.dram_tensor("biasv", [1, 1], f32, kind="ExternalInput")
    out_d = nc.dram_tensor("out", [ROWS_OUT, OW], f32, kind="ExternalOutput")

    with tile.TileContext(nc) as tc:
        with (
            tc.tile_pool(name="bp", bufs=1) as bp,
            tc.tile_pool(name="xp", bufs=1) as xp,
            tc.tile_pool(name="op", bufs=3) as op,
            tc.tile_pool(name="pp", bufs=6, space=bass.MemorySpace.PSUM) as pp,
            tc.tile_pool(name="pw", bufs=1, space=bass.MemorySpace.PSUM) as pw,
        ):
            def dma_rows(eng, dst, src, rows, nsplit):
                # split a [rows, ...] transfer into row chunks so the HWDGE
                # fans it across more DMA engines (one 16KB packet per row;
                # a single dma_start only engages ~2 engines)
                step = (rows + nsplit - 1) // nsplit
                for c0 in range(0, rows, step):
                    c1 = min(c0 + step, rows)
                    eng.dma_start(dst[c0:c1], src[c0:c1])

            # bands first on the sync queue (gates the first matmul)
            bt = bp.tile([128, KW * M_FULL], f32r, name="bt")
            dma_rows(nc.sync, bt, bd_d.ap()[:, :], 128, 4)

            # bias: one-packet DMA, then broadcast across partitions with a
            # K=1 matmul against a ones row (a 128-packet broadcast DMA is
            # ~13us; this chain is ~2us)
            bias_sb = bp.tile([1, 1], f32, name="bias_sb")
            nc.sync.dma_start(bias_sb[:], bias_d.ap()[:, :])
            ones_t = bp.tile([1, 128], f32, name="ones_t")
            nc.gpsimd.memset(ones_t[:], 1.0)
            bias_ps = pw.tile([128, 1], f32, name="bias_ps")
            nc.tensor.matmul(bias_ps[:], ones_t[:], bias_sb[:],
                             start=True, stop=True)
            bias_bc = bp.tile([128, 1], f32, name="bias_bc")
            nc.scalar.copy(bias_bc[:], bias_ps[:])

            # all slab loads issued upfront: every xt is SBUF-resident (5 x
            # 2MB), so no load trigger ever queues behind a store's wait.
            # Triggers alternate between the Sync queue and GpSimd (SWDGE)
            # so trigger issue itself is parallel; slab 0 goes first.
            xts = {}
            for si, (r0, _, _, _) in enumerate(SLABS):
                xt = xp.tile([128, W], f32r, tag=f"xt{si}", name=f"xt{si}")
                dma_rows(nc.sync, xt, xs_d.ap()[r0:r0 + 128, :], 128,
                         8 if si == 0 else 4)
                xts[si] = xt

            # warm the PE's HAM clock gate while slab 0's DMA is in flight
            warm = pw.tile([118, 512], f32, name="warm")
            for i in range(14):
                nc.tensor.matmul(warm[:, :], bt[:, 0:118], bt[:, 0:512],
                                 start=(i == 0), stop=(i == 13))

            for si, (r0, o0, M, boff) in enumerate(SLABS):
                xt = xts[si]
                ot = op.tile([M, OW], f32, tag="ot", name=f"ot{si}")
                for b in range(8):
                    n0 = b * 512
                    N = BANK_N[b]
                    pt = pp.tile([M, 512], f32, tag="ps", name=f"ps{si}_{b}")
                    for dj in range(KW):
                        nc.tensor.matmul(
                            pt[:, :N],
                            bt[:, dj * M_FULL + boff: dj * M_FULL + boff + M],
                            xt[:, n0 + dj: n0 + dj + N],
                            start=(dj == 0),
                            stop=(dj == KW - 1),
                        )
                    nc.scalar.activation(
                        ot[:, n0:n0 + N], pt[:, :N],
                        mybir.ActivationFunctionType.Identity,
                        bias=bias_bc[0:M, :],
                    )
                # stores on sync: all load triggers were already issued, so
                # a store trigger waiting on this slab's ACTs blocks nothing
                dma_rows(nc.sync, out_d.ap()[o0:o0 + M, :], ot[:], M, 4)
    nc.compile()
    return nc


def _bands_from_weight(weight: np.ndarray) -> np.ndarray:
    b = np.zeros((128, KW * M_FULL), np.float32)
    for dj in range(KW):
        col = weight[:, dj].astype(np.float32)
        for m in range(M_FULL):
            b[m:m + KH, dj * M_FULL + m] = col
    return b


def kernel(x: np.ndarray, weight: np.ndarray, bias: np.ndarray,
           _trace: bool = False, **_trace_kwargs) -> np.ndarray:
    global LAST_RESULT
    x = np.asarray(x, dtype=np.float32)
    weight = np.asarray(weight, dtype=np.float32)
    bias_v = np.asarray(bias, dtype=np.float32).reshape(1, 1)

    if "nc" not in _cache:
        _cache["nc"] = _build()
    nc = _cache["nc"]

    bands = _bands_from_weight(weight)
    starts = [min(m * ROWS_OUT, H - ROWS_IN) for m in range(NCORES)]
    in_maps = [
        {"xs": np.ascontiguousarray(x[s:s + ROWS_IN]),
         "bands": bands,
         "biasv": bias_v}
        for s in starts
    ]
    res = run_bass_kernel_spmd(nc, in_maps, core_ids=list(range(NCORES)),
                               trace=_trace, **_trace_kwargs)
    LAST_RESULT = res

    out = np.empty((OH, OW), dtype=np.float32)
    for m, s in enumerate(starts):
        r = res.results[m]["out"]
        g0 = m * ROWS_OUT           # first global output row wanted from core m
        keep0 = g0 - s              # 0 for cores 0-6, 10 for core 7
        take = min(ROWS_OUT - keep0, OH - g0)
        out[g0:g0 + take] = r[keep0:keep0 + take]
    return out
